# revision 1
# baseline (speedup 1.0000x reference)
"""Trainium2 Bass kernel for nn_Block_68719476955 (dense transformer block).

Math: with H=1 the attention softmax is over a singleton axis, so
attn_prob == 1.0 exactly and the whole attention reduces to
x @ w_kv + b_kv (w_attn / b_attn / mask do not affect the output).

The block computed per token row x_t (E=2048):
    t  = x @ w_kv + b_kv
    h  = LN(x + t) * g1 + b1
    u  = gelu(h @ w_fc + b_fc)          # exact gelu
    v  = u @ w_mproj + b_mproj
    out= LN(v + x) * g2 + b2

Distribution: pure data-parallel over the 8192 tokens across 8 cores
(1024 tokens/core), full weights on every core, no collectives.

Device layout: feature-major ("transposed") activations [E, tokens] so
every matmul runs with the weight block as the stationary operand
(lhsT = W[in,out] chunk, rhs = activation [in, tok]) and the output
lands feature-major again — zero on-device transposes. LayerNorm
reductions (over features = partitions) run on the TensorEngine as
ones-vector matmuls, software-pipelined one chunk behind the producing
matmuls; per-token stats come back across partitions via
gpsimd.partition_broadcast (no PE involvement).

Precision: bf16 matmul operands, fp32 PSUM accumulation, fp32
residual adds and final normalize.
"""

import numpy as np
import ml_dtypes
from contextlib import ExitStack

import concourse.bacc as bacc
import concourse.mybir as mybir
import concourse.tile as tile
from concourse.bass_utils import run_bass_kernel_spmd

P = 128
B, S, E = 4, 2048, 2048
H4 = 4 * E                 # 8192 mlp hidden
NCORES = 8
TOK = (B * S) // NCORES    # 1024 tokens per core
T = 512                    # token tile (2 per core)
NT = TOK // T
EO = E // P                # 16
FO = H4 // P               # 64
HC = 4                     # hidden chunks for the mlp (2048 features each)
HCO = FO // HC             # 16 m-blocks per hidden chunk
LN_EPS = 1e-5

F32 = mybir.dt.float32
BF16 = mybir.dt.bfloat16
AF = mybir.ActivationFunctionType
ALU = mybir.AluOpType

_CACHED_NC = None


def _build():
    nc = bacc.Bacc(None, target_bir_lowering=False)

    xf_d = nc.dram_tensor("xf", [E, TOK], F32, kind="ExternalInput")
    xb_d = nc.dram_tensor("xb", [E, TOK], BF16, kind="ExternalInput")
    wkv_d = nc.dram_tensor("wkv", [EO, P, EO, P], BF16, kind="ExternalInput")
    wfc_d = nc.dram_tensor("wfc", [FO, P, EO, P], BF16, kind="ExternalInput")
    wmp_d = nc.dram_tensor("wmp", [EO, P, FO, P], BF16, kind="ExternalInput")
    bkv_d = nc.dram_tensor("bkv", [P, EO], F32, kind="ExternalInput")
    bfc_d = nc.dram_tensor("bfc", [P, FO], F32, kind="ExternalInput")
    bmp_d = nc.dram_tensor("bmp", [P, EO], F32, kind="ExternalInput")
    g1_d = nc.dram_tensor("g1", [P, EO], F32, kind="ExternalInput")
    b1_d = nc.dram_tensor("b1", [P, EO], F32, kind="ExternalInput")
    g2_d = nc.dram_tensor("g2", [P, EO], F32, kind="ExternalInput")
    b2_d = nc.dram_tensor("b2", [P, EO], F32, kind="ExternalInput")
    out_d = nc.dram_tensor("out", [E, TOK], F32, kind="ExternalOutput")

    with tile.TileContext(nc) as tc, ExitStack() as ctx:
        consts = ctx.enter_context(tc.tile_pool(name="consts", bufs=1))
        xbp = ctx.enter_context(tc.tile_pool(name="xbp", bufs=1))
        wp = ctx.enter_context(tc.tile_pool(name="wp", bufs=3))
        xcp = ctx.enter_context(tc.tile_pool(name="xcp", bufs=2))
        rbp = ctx.enter_context(tc.tile_pool(name="rbp", bufs=2))
        up = ctx.enter_context(tc.tile_pool(name="up", bufs=1))
        vp = ctx.enter_context(tc.tile_pool(name="vp", bufs=1))
        tmp = ctx.enter_context(tc.tile_pool(name="tmp", bufs=4))
        sqp = ctx.enter_context(tc.tile_pool(name="sqp", bufs=8))
        stp = ctx.enter_context(tc.tile_pool(name="stp", bufs=1))
        bcp = ctx.enter_context(tc.tile_pool(name="bcp", bufs=2))
        psmm = ctx.enter_context(tc.tile_pool(name="psmm", bufs=4, space="PSUM"))
        psst = ctx.enter_context(tc.tile_pool(name="psst", bufs=2, space="PSUM"))

        # x in bf16, one tile per 128-feature chunk so the kv matmuls can
        # start as soon as their chunk lands (16 parallel DMAs, issued first).
        xbs = []
        for k in range(EO):
            xk = xbp.tile([P, TOK], BF16, tag=f"xb{k}")
            eng = nc.gpsimd if k % 2 == 0 else nc.scalar
            eng.dma_start(xk[:], xb_d[k * P:(k + 1) * P, :])
            xbs.append(xk)

        # --- constants (gpsimd queue keeps Sync free for the weight stream) ---
        bkv_t = consts.tile([P, EO], F32)
        nc.gpsimd.dma_start(bkv_t[:], bkv_d[:, :])
        bfc_t = consts.tile([P, FO], F32)
        nc.gpsimd.dma_start(bfc_t[:], bfc_d[:, :])
        bmp_t = consts.tile([P, EO], F32)
        nc.gpsimd.dma_start(bmp_t[:], bmp_d[:, :])
        g1_t = consts.tile([P, EO], F32)
        nc.gpsimd.dma_start(g1_t[:], g1_d[:, :])
        b1_t = consts.tile([P, EO], F32)
        nc.gpsimd.dma_start(b1_t[:], b1_d[:, :])
        g2_t = consts.tile([P, EO], F32)
        nc.gpsimd.dma_start(g2_t[:], g2_d[:, :])
        b2_t = consts.tile([P, EO], F32)
        nc.gpsimd.dma_start(b2_t[:], b2_d[:, :])
        ones_col = consts.tile([P, 1], BF16)
        nc.vector.memset(ones_col[:], 1.0)
        eps_t = consts.tile([1, 1], F32)
        nc.vector.memset(eps_t[:], LN_EPS)

        # warm the PE (HAM clock gate) while input DMAs are in flight
        warm_rhs = consts.tile([P, T], BF16)
        nc.vector.memset(warm_rhs[:], 1.0)
        warm_ps = psst.tile([1, T], F32, tag="pss")
        for _ in range(40):
            nc.tensor.matmul(warm_ps[:], lhsT=ones_col[:], rhs=warm_rhs[:],
                             start=True, stop=True)

        def tsl(t):
            return slice(t * T, (t + 1) * T)

        def ln_finalize(ps_sum, ps_sq):
            """per-token mean/var from accumulated sums -> [P,2,T] bcast
            (slot 0 = rstd, slot 1 = mean*rstd). PE-free."""
            st = stp.tile([1, 3, T], F32, tag="st")
            nc.vector.tensor_scalar_mul(st[:, 0, :], ps_sum[:], 1.0 / E)   # mean
            nc.vector.tensor_scalar_mul(st[:, 1, :], ps_sq[:], 1.0 / E)    # E[x^2]
            nc.vector.tensor_mul(out=st[:, 2, :], in0=st[:, 0, :], in1=st[:, 0, :])
            nc.vector.tensor_tensor(st[:, 1, :], st[:, 1, :], st[:, 2, :],
                                    ALU.subtract)                          # var
            nc.scalar.activation(st[:, 2, :], st[:, 1, :], AF.Sqrt,
                                 bias=eps_t[:], scale=1.0)                 # sqrt(var+eps)
            nc.vector.reciprocal(out=st[:, 2, :], in_=st[:, 2, :])         # rstd
            nc.vector.tensor_mul(out=st[:, 1, :], in0=st[:, 0, :], in1=st[:, 2, :])
            # slots: [1]=mean*rstd, [2]=rstd -> bcast adjacent pair
            bc = bcp.tile([P, 2, T], F32, tag="bc")
            nc.gpsimd.partition_broadcast(bc[:], st[:, 1:3, :])
            return bc

        # ---------- phase A: kv matmul + residual + LN1 stats ----------
        def phase_a(t, hook=None, block_cb=None):
            r1b = rbp.tile([P, EO, T], BF16, tag="rb")
            ps_sum = psst.tile([1, T], F32, tag="pss")
            ps_sq = psst.tile([1, T], F32, tag="psq")
            pending = []  # software-pipeline the stats MMs behind the k-loops

            def emit_stats(pi):
                # pair-sum chunks 2pi,2pi+1 on DVE, then one ones-MM per pair
                m0, m1 = 2 * pi, 2 * pi + 1
                pr = sqp.tile([P, T], BF16, tag="sq")
                nc.vector.tensor_add(out=pr[:], in0=r1b[:, m0, :],
                                     in1=r1b[:, m1, :])
                sq0 = pending.pop(0)
                sq1 = pending.pop(0)
                pq = sqp.tile([P, T], BF16, tag="sq")
                nc.vector.tensor_add(out=pq[:], in0=sq0[:], in1=sq1[:])
                nc.tensor.matmul(ps_sum[:], lhsT=ones_col[:], rhs=pr[:],
                                 start=(pi == 0), stop=(pi == EO // 2 - 1))
                nc.tensor.matmul(ps_sq[:], lhsT=ones_col[:], rhs=pq[:],
                                 start=(pi == 0), stop=(pi == EO // 2 - 1))

            for m in range(EO):
                wt = wp.tile([P, EO, P], BF16, tag="w")
                nc.sync.dma_start(wt[:], wkv_d[m])
                ps = psmm.tile([P, T], F32, tag="ps")
                for k in range(EO):
                    nc.tensor.matmul(ps[:], lhsT=wt[:, k, :],
                                     rhs=xbs[k][:, tsl(t)],
                                     start=(k == 0), stop=(k == EO - 1))
                if m == 0 and hook is not None:
                    hook()  # previous phase's deferred stats MMs
                xc = xcp.tile([P, T], F32, tag="xc")
                nc.sync.dma_start(xc[:], xf_d[m * P:(m + 1) * P, tsl(t)])
                t1 = tmp.tile([P, T], F32, tag="t1")
                nc.scalar.activation(t1[:], ps[:], AF.Identity,
                                     bias=bkv_t[:, m:m + 1], scale=1.0)
                nc.vector.tensor_add(out=r1b[:, m, :], in0=t1[:], in1=xc[:])
                sq = sqp.tile([P, T], BF16, tag="sq")
                nc.vector.tensor_mul(out=sq[:], in0=r1b[:, m, :], in1=r1b[:, m, :])
                pending.append(sq)
                if m % 4 == 3 and m < EO - 1:
                    emit_stats(m // 2 - 1)
                    emit_stats(m // 2)
                if block_cb is not None:
                    block_cb(m)

            def finish():
                emit_stats(EO // 2 - 2)
                emit_stats(EO // 2 - 1)
            return r1b, (ps_sum, ps_sq), finish

        def normalize1_chunk(r1b, bc, m):
            # in place: h overwrites r1b (WAR on the stats MMs is tracked)
            t1 = tmp.tile([P, T], F32, tag="t1")
            nc.vector.tensor_mul(out=t1[:], in0=r1b[:, m, :], in1=bc[:, 1, :])
            nc.vector.tensor_tensor(t1[:], t1[:], bc[:, 0, :], ALU.subtract)
            nc.scalar.activation(r1b[:, m, :], t1[:], AF.Identity,
                                 bias=b1_t[:, m:m + 1],
                                 scale=g1_t[:, m:m + 1])

        def normalize1(r1b, bc):
            for m in range(EO):
                normalize1_chunk(r1b, bc, m)
            return r1b

        # ---------- phase B: mlp; LN2 stats fused into last-chunk evicts ----
        def phase_b(t, h, hook=None):
            v2f = vp.tile([P, EO, T], F32, tag="v")     # r2 = v + b_mproj + x
            ps_sum = psst.tile([1, T], F32, tag="pss")
            ps_sq = psst.tile([1, T], F32, tag="psq")
            pending = []

            def emit_stats2(pi):
                r2c0, sq0 = pending.pop(0)
                r2c1, sq1 = pending.pop(0)
                pr = sqp.tile([P, T], BF16, tag="sq")
                nc.vector.tensor_add(out=pr[:], in0=r2c0[:], in1=r2c1[:])
                pq = sqp.tile([P, T], BF16, tag="sq")
                nc.vector.tensor_add(out=pq[:], in0=sq0[:], in1=sq1[:])
                nc.tensor.matmul(ps_sum[:], lhsT=ones_col[:], rhs=pr[:],
                                 start=(pi == 0), stop=(pi == EO // 2 - 1))
                nc.tensor.matmul(ps_sq[:], lhsT=ones_col[:], rhs=pq[:],
                                 start=(pi == 0), stop=(pi == EO // 2 - 1))

            u = up.tile([P, FO, T], BF16, tag="u")
            for ma in range(FO):
                wt = wp.tile([P, EO, P], BF16, tag="w")
                nc.sync.dma_start(wt[:], wfc_d[ma])
                ps = psmm.tile([P, T], F32, tag="ps")
                for k in range(EO):
                    nc.tensor.matmul(ps[:], lhsT=wt[:, k, :], rhs=h[:, k, :],
                                     start=(k == 0), stop=(k == EO - 1))
                if ma == 0 and hook is not None:
                    hook()  # previous phase's deferred stats + LN2 output
                nc.scalar.activation(u[:, ma, :], ps[:], AF.Gelu,
                                     bias=bfc_t[:, ma:ma + 1], scale=1.0)
            for mo in range(EO):
                ps = psmm.tile([P, T], F32, tag="ps")
                for hc in range(HC):
                    wt = wp.tile([P, HCO, P], BF16, tag="w")
                    nc.sync.dma_start(wt[:], wmp_d[mo][:, hc * HCO:(hc + 1) * HCO, :])
                    for k in range(HCO):
                        nc.tensor.matmul(ps[:], lhsT=wt[:, k, :],
                                         rhs=u[:, hc * HCO + k, :],
                                         start=(hc == 0 and k == 0),
                                         stop=(hc == HC - 1 and k == HCO - 1))
                xc = xcp.tile([P, T], F32, tag="xc")
                nc.gpsimd.dma_start(xc[:], xf_d[mo * P:(mo + 1) * P, tsl(t)])
                nc.scalar.activation(v2f[:, mo, :], ps[:], AF.Identity,
                                     bias=bmp_t[:, mo:mo + 1], scale=1.0)
                nc.vector.tensor_add(out=v2f[:, mo, :], in0=v2f[:, mo, :],
                                     in1=xc[:])
                r2c = sqp.tile([P, T], BF16, tag="sq")
                nc.vector.tensor_copy(out=r2c[:], in_=v2f[:, mo, :])
                sq = sqp.tile([P, T], BF16, tag="sq")
                nc.vector.tensor_mul(out=sq[:], in0=r2c[:], in1=r2c[:])
                pending.append((r2c, sq))
                if mo >= 2 and mo % 2 == 0:
                    emit_stats2(mo // 2 - 1)

            def finish():
                emit_stats2(EO // 2 - 1)
            return v2f, (ps_sum, ps_sq), finish

        # ---------- phase C: final normalize + output ----------
        def phase_c_out(t, v2f, bc, tail):
            dma_eng = nc.sync if tail else nc.gpsimd
            for m in range(EO):
                t1 = tmp.tile([P, T], F32, tag="t1")
                nc.vector.tensor_mul(out=t1[:], in0=v2f[:, m, :], in1=bc[:, 1, :])
                nc.vector.tensor_tensor(t1[:], t1[:], bc[:, 0, :], ALU.subtract)
                if tail:
                    nc.scalar.activation(t1[:], t1[:], AF.Identity,
                                         bias=b2_t[:, m:m + 1],
                                         scale=g2_t[:, m:m + 1])
                else:
                    nc.vector.tensor_scalar(t1[:], t1[:], g2_t[:, m:m + 1],
                                            b2_t[:, m:m + 1], ALU.mult, ALU.add)
                dma_eng.dma_start(out_d[m * P:(m + 1) * P, tsl(t)], t1[:])

        # Emission order interleaves the two token tiles so the PE never
        # waits on DVE normalize chains or LN finalize chains; each phase's
        # final stats MMs are deferred into the next phase's first block.
        r1b0, s0, f0 = phase_a(0)
        state = {}

        def hook_a1():
            f0()
            state["bc10"] = ln_finalize(*s0)

        def a1_block_cb(m):
            normalize1_chunk(r1b0, state["bc10"], m)

        r1b1, s1, f1 = phase_a(1, hook=hook_a1, block_cb=a1_block_cb)

        def hook_b0():
            f1()
            bc11 = ln_finalize(*s1)
            normalize1(r1b1, bc11)

        v0, s20, f20 = phase_b(0, r1b0, hook=hook_b0)

        def hook_b1():
            f20()
            bc20 = ln_finalize(*s20)
            phase_c_out(0, v0, bc20, tail=False)

        v1, s21, f21 = phase_b(1, r1b1, hook=hook_b1)
        f21()
        bc21 = ln_finalize(*s21)
        phase_c_out(1, v1, bc21, tail=True)

    nc.compile()
    return nc


def _get_nc():
    global _CACHED_NC
    if _CACHED_NC is None:
        _CACHED_NC = _build()
    return _CACHED_NC


def _prep_inputs(x, w_kv, b_kv, w_fc, b_fc, w_mproj, b_mproj,
                 ln1_g, ln1_b, ln2_g, ln2_b):
    """Host-side shard + retile. Returns per-core input maps."""
    bf = ml_dtypes.bfloat16
    x_flat = np.ascontiguousarray(np.asarray(x, dtype=np.float32).reshape(B * S, E))

    # weights: [in, out] -> [m, p, o, c] tiles, in = o*128+p, out = m*128+c
    def retile(w, io, oo):
        w = np.asarray(w, dtype=np.float32).reshape(io, P, oo, P)
        return np.ascontiguousarray(w.transpose(2, 1, 0, 3)).astype(bf)

    wkv_t = retile(w_kv, EO, EO)
    wfc_t = retile(w_fc, EO, FO)
    wmp_t = retile(w_mproj, FO, EO)

    def p2d(v):  # [n*P] -> [P, n] with chunk o in column o
        v = np.asarray(v, np.float32)
        return np.ascontiguousarray(v.reshape(-1, P).T)

    shared = {
        "wkv": wkv_t, "wfc": wfc_t, "wmp": wmp_t,
        "bkv": p2d(b_kv), "bfc": p2d(b_fc), "bmp": p2d(b_mproj),
        "g1": p2d(ln1_g), "b1": p2d(ln1_b), "g2": p2d(ln2_g), "b2": p2d(ln2_b),
    }
    in_maps = []
    for c in range(NCORES):
        xT = np.ascontiguousarray(x_flat[c * TOK:(c + 1) * TOK].T)  # [E, TOK] f32
        in_maps.append({"xf": xT, "xb": xT.astype(bf), **shared})
    return in_maps


def _run(inputs, trace=False):
    nc = _get_nc()
    in_maps = _prep_inputs(
        inputs["x"], inputs["w_kv"], inputs["b_kv"], inputs["w_fc"],
        inputs["b_fc"], inputs["w_mproj"], inputs["b_mproj"],
        inputs["ln1_g"], inputs["ln1_b"], inputs["ln2_g"], inputs["ln2_b"])
    res = run_bass_kernel_spmd(nc, in_maps, core_ids=list(range(NCORES)),
                               trace=trace)
    outs = [np.asarray(res.results[c]["out"], dtype=np.float32).T
            for c in range(NCORES)]
    full = np.concatenate(outs, axis=0).reshape(B, S, E)
    return full, res


def kernel(**inputs) -> np.ndarray:
    out, _ = _run(inputs, trace=False)
    return out



# revision 6
# speedup vs baseline: 1.1115x; 1.1115x over previous
"""Trainium2 Bass kernel for nn_Block_68719476955 (dense transformer block).

Math: with H=1 the attention softmax is over a singleton axis, so
attn_prob == 1.0 exactly and the whole attention reduces to
x @ w_kv + b_kv (w_attn / b_attn / mask do not affect the output).

The block computed per token row x_t (E=2048):
    t  = x @ w_kv + b_kv
    h  = LN(x + t) * g1 + b1
    u  = gelu(h @ w_fc + b_fc)          # exact gelu
    v  = u @ w_mproj + b_mproj
    out= LN(v + x) * g2 + b2

Distribution: pure data-parallel over the 8192 tokens across 8 cores
(1024 tokens/core), full weights on every core, no collectives.

Device layout: feature-major ("transposed") activations [E, tokens] so
every matmul runs with the weight block as the stationary operand
(lhsT = W[in,out] chunk, rhs = activation [in, tok]) and the output
lands feature-major again — zero on-device transposes. LayerNorm
reductions (over features = partitions) run on the TensorEngine as
ones-vector matmuls (chunk tiles pre-summed in quads on DVE), one
chunk behind the producing matmuls; per-token stats come back across
partitions via gpsimd.partition_broadcast.

Precision: bf16 matmul operands with fp32 PSUM accumulation, except a
slice of the mproj contraction (F8MP of 64 hidden chunks) and of the
fc contraction (F8FC of 16 h chunks) which run as fp8-e4m3 DoubleRow
pair-matmuls (2x MAC rate). All products within one PSUM group share
one power-of-2 scale: the fp8 path quantizes act*sa / weight*64, the
bf16 path pre-scales its weights by sa*64 on the host, and the
eviction divides by sa*64. Measured end-to-end rel err ~1.5e-2 vs the
2e-2 gate (bf16-only is ~3e-3; each fp8 chunk adds sqrt(f)-scaled
quantization noise).
"""

import numpy as np
import ml_dtypes
from contextlib import ExitStack

import concourse.bacc as bacc
import concourse.mybir as mybir
import concourse.tile as tile
from concourse.bass_utils import run_bass_kernel_spmd

P = 128
B, S, E = 4, 2048, 2048
H4 = 4 * E                 # 8192 mlp hidden
NCORES = 8
TOK = (B * S) // NCORES    # 1024 tokens per core
T = 512                    # token tile (2 per core)
NT = TOK // T
EO = E // P                # 16
FO = H4 // P               # 64
LN_EPS = 1e-5

# fp8 fractions: last F8MP of the 64 mproj k-chunks and last F8FC of the
# 16 fc k-chunks run as DoubleRow fp8 pairs (must be even).
F8MP = 20
F8FC = 0
BFMP = FO - F8MP
BFFC = EO - F8FC
SA_H = 8.0                 # h quantization scale for fc fp8 path
SW = 64.0                  # weight quantization scale for fp8 paths
MP_SCALE = SW              # mproj products: u(x1) * w(x64)
FC_SCALE = (SA_H * SW) if F8FC else 1.0

F32 = mybir.dt.float32
BF16 = mybir.dt.bfloat16
F8 = mybir.dt.float8e4
DRM = mybir.MatmulPerfMode.DoubleRow
AF = mybir.ActivationFunctionType
ALU = mybir.AluOpType
E4NP = ml_dtypes.float8_e4m3

_CACHED_NC = None


def _build():
    nc = bacc.Bacc(None, target_bir_lowering=False)

    xf_d = nc.dram_tensor("xf", [E, TOK], F32, kind="ExternalInput")
    xb_d = nc.dram_tensor("xb", [E, TOK], BF16, kind="ExternalInput")
    wkv_d = nc.dram_tensor("wkv", [EO, P, EO, P], BF16, kind="ExternalInput")
    wfc_d = nc.dram_tensor("wfc", [FO, P, BFFC, P], BF16, kind="ExternalInput")
    wmp_d = nc.dram_tensor("wmp", [EO, P, BFMP, P], BF16, kind="ExternalInput")
    wmp8_d = nc.dram_tensor("wmp8", [EO, P, max(F8MP // 2, 1), 2, P], F8,
                            kind="ExternalInput")
    wfc8_d = nc.dram_tensor("wfc8", [FO, P, max(F8FC // 2, 1), 2, P], F8,
                            kind="ExternalInput")
    bkv_d = nc.dram_tensor("bkv", [P, EO], F32, kind="ExternalInput")
    bfc_d = nc.dram_tensor("bfc", [P, FO], F32, kind="ExternalInput")
    bmp_d = nc.dram_tensor("bmp", [P, EO], F32, kind="ExternalInput")
    g1_d = nc.dram_tensor("g1", [P, EO], F32, kind="ExternalInput")
    b1_d = nc.dram_tensor("b1", [P, EO], F32, kind="ExternalInput")
    g1s_d = nc.dram_tensor("g1s", [P, EO], F32, kind="ExternalInput")
    b1s_d = nc.dram_tensor("b1s", [P, EO], F32, kind="ExternalInput")
    g2_d = nc.dram_tensor("g2", [P, EO], F32, kind="ExternalInput")
    b2_d = nc.dram_tensor("b2", [P, EO], F32, kind="ExternalInput")
    out_d = nc.dram_tensor("out", [E, TOK], F32, kind="ExternalOutput")

    with tile.TileContext(nc) as tc, ExitStack() as ctx:
        consts = ctx.enter_context(tc.tile_pool(name="consts", bufs=1))
        xbp = ctx.enter_context(tc.tile_pool(name="xbp", bufs=1))
        wp = ctx.enter_context(tc.tile_pool(name="wp", bufs=3))
        w8p = ctx.enter_context(tc.tile_pool(name="w8p", bufs=2))
        xcp = ctx.enter_context(tc.tile_pool(name="xcp", bufs=2))
        rbp = ctx.enter_context(tc.tile_pool(name="rbp", bufs=2))
        h8p = ctx.enter_context(tc.tile_pool(name="h8p", bufs=2))
        up = ctx.enter_context(tc.tile_pool(name="up", bufs=1))
        vp = ctx.enter_context(tc.tile_pool(name="vp", bufs=1))
        tmp = ctx.enter_context(tc.tile_pool(name="tmp", bufs=4))
        sqp = ctx.enter_context(tc.tile_pool(name="sqp", bufs=8))
        stp = ctx.enter_context(tc.tile_pool(name="stp", bufs=1))
        bcp = ctx.enter_context(tc.tile_pool(name="bcp", bufs=2))
        psmm = ctx.enter_context(tc.tile_pool(name="psmm", bufs=4, space="PSUM"))
        psst = ctx.enter_context(tc.tile_pool(name="psst", bufs=2, space="PSUM"))

        # x in bf16, one tile per 128-feature chunk so the kv matmuls can
        # start as soon as their chunk lands (16 parallel DMAs, issued first).
        xbs = []
        for k in range(EO):
            xk = xbp.tile([P, TOK], BF16, tag=f"xb{k}")
            eng = nc.gpsimd if k % 2 == 0 else nc.scalar
            eng.dma_start(xk[:], xb_d[k * P:(k + 1) * P, :])
            xbs.append(xk)

        # --- constants (gpsimd queue keeps Sync free for the weight stream) ---
        bkv_t = consts.tile([P, EO], F32)
        nc.gpsimd.dma_start(bkv_t[:], bkv_d[:, :])
        bfc_t = consts.tile([P, FO], F32)
        nc.gpsimd.dma_start(bfc_t[:], bfc_d[:, :])
        bmp_t = consts.tile([P, EO], F32)
        nc.gpsimd.dma_start(bmp_t[:], bmp_d[:, :])
        g1_t = consts.tile([P, EO], F32)
        nc.gpsimd.dma_start(g1_t[:], g1_d[:, :])
        b1_t = consts.tile([P, EO], F32)
        nc.gpsimd.dma_start(b1_t[:], b1_d[:, :])
        g1s_t = consts.tile([P, EO], F32)
        nc.gpsimd.dma_start(g1s_t[:], g1s_d[:, :])
        b1s_t = consts.tile([P, EO], F32)
        nc.gpsimd.dma_start(b1s_t[:], b1s_d[:, :])
        g2_t = consts.tile([P, EO], F32)
        nc.gpsimd.dma_start(g2_t[:], g2_d[:, :])
        b2_t = consts.tile([P, EO], F32)
        nc.gpsimd.dma_start(b2_t[:], b2_d[:, :])
        ones_col = consts.tile([P, 1], BF16)
        nc.vector.memset(ones_col[:], 1.0)
        eps_t = consts.tile([1, 1], F32)
        nc.vector.memset(eps_t[:], LN_EPS)

        # warm the PE (HAM clock gate) while input DMAs are in flight
        warm_rhs = consts.tile([P, T], BF16)
        nc.vector.memset(warm_rhs[:], 1.0)
        warm_ps = psst.tile([1, T], F32, tag="pss")
        for _ in range(16):
            nc.tensor.matmul(warm_ps[:], lhsT=ones_col[:], rhs=warm_rhs[:],
                             start=True, stop=True)

        def tsl(t):
            return slice(t * T, (t + 1) * T)

        def ln_finalize(ps_sum, ps_sq):
            """per-token mean/var from accumulated sums -> [P,2,T] bcast
            (slot 0 = mean*rstd, slot 1 = rstd). PE-free."""
            st = stp.tile([1, 3, T], F32, tag="st")
            nc.vector.tensor_scalar_mul(st[:, 0, :], ps_sum[:], 1.0 / E)   # mean
            nc.vector.tensor_scalar_mul(st[:, 1, :], ps_sq[:], 1.0 / E)    # E[x^2]
            nc.vector.tensor_mul(out=st[:, 2, :], in0=st[:, 0, :], in1=st[:, 0, :])
            nc.vector.tensor_tensor(st[:, 1, :], st[:, 1, :], st[:, 2, :],
                                    ALU.subtract)                          # var
            nc.scalar.activation(st[:, 2, :], st[:, 1, :], AF.Sqrt,
                                 bias=eps_t[:], scale=1.0)                 # sqrt(var+eps)
            nc.vector.reciprocal(out=st[:, 2, :], in_=st[:, 2, :])         # rstd
            nc.vector.tensor_mul(out=st[:, 1, :], in0=st[:, 0, :], in1=st[:, 2, :])
            # slots: [1]=mean*rstd, [2]=rstd -> bcast adjacent pair
            bc = bcp.tile([P, 2, T], F32, tag="bc")
            nc.gpsimd.partition_broadcast(bc[:], st[:, 1:3, :])
            return bc

        # ---------- phase A: kv matmul + residual + LN1 stats ----------
        def phase_a(t, hook=None, block_cb=None):
            r1b = rbp.tile([P, EO, T], BF16, tag="rb")
            ps_sum = psst.tile([1, T], F32, tag="pss")
            ps_sq = psst.tile([1, T], F32, tag="psq")
            pending = []  # software-pipeline the stats MMs behind the k-loops

            def emit_stats(qi):
                # quad-sum chunks 4qi..4qi+3 on DVE (in place), then one
                # ones-MM per quad quantity
                m0 = 4 * qi
                pr = sqp.tile([P, T], BF16, tag="sq")
                nc.vector.tensor_add(out=pr[:], in0=r1b[:, m0, :],
                                     in1=r1b[:, m0 + 1, :])
                nc.vector.tensor_tensor(pr[:], pr[:], r1b[:, m0 + 2, :], ALU.add)
                nc.vector.tensor_tensor(pr[:], pr[:], r1b[:, m0 + 3, :], ALU.add)
                sq0, sq1, sq2, sq3 = (pending.pop(0) for _ in range(4))
                pq = sqp.tile([P, T], BF16, tag="sq")
                nc.vector.tensor_add(out=pq[:], in0=sq0[:], in1=sq1[:])
                nc.vector.tensor_tensor(pq[:], pq[:], sq2[:], ALU.add)
                nc.vector.tensor_tensor(pq[:], pq[:], sq3[:], ALU.add)
                nc.tensor.matmul(ps_sum[:], lhsT=ones_col[:], rhs=pr[:],
                                 start=(qi == 0), stop=(qi == EO // 4 - 1))
                nc.tensor.matmul(ps_sq[:], lhsT=ones_col[:], rhs=pq[:],
                                 start=(qi == 0), stop=(qi == EO // 4 - 1))

            for m in range(EO):
                wt = wp.tile([P, EO, P], BF16, tag="w")
                nc.sync.dma_start(wt[:], wkv_d[m])
                ps = psmm.tile([P, T], F32, tag="ps")
                for k in range(EO):
                    nc.tensor.matmul(ps[:], lhsT=wt[:, k, :],
                                     rhs=xbs[k][:, tsl(t)],
                                     start=(k == 0), stop=(k == EO - 1))
                if m == 0 and hook is not None:
                    hook()  # previous phase's deferred stats MMs
                xc = xcp.tile([P, T], F32, tag="xc")
                nc.sync.dma_start(xc[:], xf_d[m * P:(m + 1) * P, tsl(t)])
                t1 = tmp.tile([P, T], F32, tag="t1")
                nc.scalar.activation(t1[:], ps[:], AF.Identity,
                                     bias=bkv_t[:, m:m + 1], scale=1.0)
                nc.vector.tensor_add(out=r1b[:, m, :], in0=t1[:], in1=xc[:])
                sq = sqp.tile([P, T], BF16, tag="sq")
                nc.vector.tensor_mul(out=sq[:], in0=r1b[:, m, :], in1=r1b[:, m, :])
                pending.append(sq)
                if m % 4 == 3 and m < EO - 1:
                    emit_stats(m // 4)
                if block_cb is not None:
                    block_cb(m)

            def finish():
                emit_stats(3)
            return r1b, (ps_sum, ps_sq), finish

        def normalize1_chunk(r1b, h8, bc, m):
            # in place: h overwrites r1b (WAR on the stats MMs is tracked);
            # the last F8FC chunks also get an fp8 copy at scale SA_H.
            t1 = tmp.tile([P, T], F32, tag="t1")
            nc.vector.tensor_mul(out=t1[:], in0=r1b[:, m, :], in1=bc[:, 1, :])
            nc.vector.tensor_tensor(t1[:], t1[:], bc[:, 0, :], ALU.subtract)
            if m >= BFFC:
                nc.scalar.activation(h8[:, m - BFFC, :], t1[:], AF.Identity,
                                     bias=b1s_t[:, m:m + 1],
                                     scale=g1s_t[:, m:m + 1])
            if m < BFFC:
                nc.scalar.activation(r1b[:, m, :], t1[:], AF.Identity,
                                     bias=b1_t[:, m:m + 1],
                                     scale=g1_t[:, m:m + 1])

        def normalize1(r1b, h8, bc):
            for m in range(EO):
                normalize1_chunk(r1b, h8, bc, m)

        # ---------- phase B: mlp; LN2 stats fused into last-chunk evicts ----
        def phase_b(t, h, h8, hook=None):
            v2f = vp.tile([P, EO, T], F32, tag="v")     # r2 = v + b_mproj + x
            ps_sum = psst.tile([1, T], F32, tag="pss")
            ps_sq = psst.tile([1, T], F32, tag="psq")
            pending = []

            def emit_stats2(qi):
                # r2 chunk sums read straight from v2f (f32) with bf16 out
                m0 = 4 * qi
                s0, s1, s2, s3 = (pending.pop(0) for _ in range(4))
                pr = sqp.tile([P, T], BF16, tag="sq")
                nc.vector.tensor_add(out=pr[:], in0=v2f[:, m0, :],
                                     in1=v2f[:, m0 + 1, :])
                nc.vector.tensor_tensor(pr[:], pr[:], v2f[:, m0 + 2, :], ALU.add)
                nc.vector.tensor_tensor(pr[:], pr[:], v2f[:, m0 + 3, :], ALU.add)
                pq = sqp.tile([P, T], BF16, tag="sq")
                nc.vector.tensor_add(out=pq[:], in0=s0[:], in1=s1[:])
                nc.vector.tensor_tensor(pq[:], pq[:], s2[:], ALU.add)
                nc.vector.tensor_tensor(pq[:], pq[:], s3[:], ALU.add)
                nc.tensor.matmul(ps_sum[:], lhsT=ones_col[:], rhs=pr[:],
                                 start=(qi == 0), stop=(qi == EO // 4 - 1))
                nc.tensor.matmul(ps_sq[:], lhsT=ones_col[:], rhs=pq[:],
                                 start=(qi == 0), stop=(qi == EO // 4 - 1))

            ub = up.tile([P, BFMP, T], BF16, tag="u")
            u8 = up.tile([P, max(F8MP, 1), T], F8, tag="u8")
            for ma in range(FO):
                wt = wp.tile([P, BFFC, P], BF16, tag="w")
                nc.sync.dma_start(wt[:], wfc_d[ma])
                if F8FC:
                    wt8 = w8p.tile([P, F8FC // 2, 2, P], F8, tag="w8")
                    nc.sync.dma_start(wt8[:], wfc8_d[ma])
                ps = psmm.tile([P, T], F32, tag="ps")
                for k in range(BFFC):
                    nc.tensor.matmul(ps[:], lhsT=wt[:, k, :], rhs=h[:, k, :],
                                     start=(k == 0), stop=(F8FC == 0 and k == BFFC - 1))
                for j in range(F8FC // 2):
                    nc.tensor.matmul(ps[:], lhsT=wt8[:, j, :, :],
                                     rhs=h8[:, 2 * j:2 * j + 2, :],
                                     start=False, stop=(j == F8FC // 2 - 1),
                                     perf_mode=DRM)
                if ma == 0 and hook is not None:
                    hook()  # previous phase's deferred stats + LN2 output
                if ma < BFMP:
                    nc.scalar.activation(ub[:, ma, :], ps[:], AF.Gelu,
                                         bias=bfc_t[:, ma:ma + 1],
                                         scale=1.0 / FC_SCALE)
                else:
                    nc.scalar.activation(u8[:, ma - BFMP, :], ps[:], AF.Gelu,
                                         bias=bfc_t[:, ma:ma + 1],
                                         scale=1.0 / FC_SCALE)
            for mo in range(EO):
                ps = psmm.tile([P, T], F32, tag="ps")
                done = 0
                for nk in (16, 16, BFMP - 32):
                    wt = wp.tile([P, nk, P], BF16, tag="w")
                    nc.sync.dma_start(wt[:], wmp_d[mo][:, done:done + nk, :])
                    for k in range(nk):
                        nc.tensor.matmul(ps[:], lhsT=wt[:, k, :],
                                         rhs=ub[:, done + k, :],
                                         start=(done + k == 0), stop=False)
                    done += nk
                wt8 = w8p.tile([P, F8MP // 2, 2, P], F8, tag="w8")
                nc.sync.dma_start(wt8[:], wmp8_d[mo])
                for j in range(F8MP // 2):
                    nc.tensor.matmul(ps[:], lhsT=wt8[:, j, :, :],
                                     rhs=u8[:, 2 * j:2 * j + 2, :],
                                     start=False, stop=(j == F8MP // 2 - 1),
                                     perf_mode=DRM)
                xc = xcp.tile([P, T], F32, tag="xc")
                nc.gpsimd.dma_start(xc[:], xf_d[mo * P:(mo + 1) * P, tsl(t)])
                nc.scalar.activation(v2f[:, mo, :], ps[:], AF.Identity,
                                     bias=bmp_t[:, mo:mo + 1],
                                     scale=1.0 / MP_SCALE)
                nc.vector.tensor_add(out=v2f[:, mo, :], in0=v2f[:, mo, :],
                                     in1=xc[:])
                sq = sqp.tile([P, T], BF16, tag="sq")
                nc.vector.tensor_mul(out=sq[:], in0=v2f[:, mo, :],
                                     in1=v2f[:, mo, :])
                pending.append(sq)
                if mo % 4 == 3 and mo < EO - 1:
                    emit_stats2(mo // 4)

            def finish():
                emit_stats2(3)
            return v2f, (ps_sum, ps_sq), finish

        # ---------- phase C: final normalize + output ----------
        def phase_c_out(t, v2f, bc, tail):
            dma_eng = nc.sync if tail else nc.gpsimd
            for m in range(EO):
                t1 = tmp.tile([P, T], F32, tag="t1")
                nc.vector.tensor_mul(out=t1[:], in0=v2f[:, m, :], in1=bc[:, 1, :])
                nc.vector.tensor_tensor(t1[:], t1[:], bc[:, 0, :], ALU.subtract)
                if tail:
                    nc.scalar.activation(t1[:], t1[:], AF.Identity,
                                         bias=b2_t[:, m:m + 1],
                                         scale=g2_t[:, m:m + 1])
                else:
                    nc.vector.tensor_scalar(t1[:], t1[:], g2_t[:, m:m + 1],
                                            b2_t[:, m:m + 1], ALU.mult, ALU.add)
                dma_eng.dma_start(out_d[m * P:(m + 1) * P, tsl(t)], t1[:])

        # Emission order interleaves the two token tiles so the PE never
        # waits on DVE normalize chains or LN finalize chains; each phase's
        # final stats MMs are deferred into the next phase's first block.
        h8_0 = h8p.tile([P, max(F8FC, 1), T], F8, tag="h8")
        h8_1 = h8p.tile([P, max(F8FC, 1), T], F8, tag="h8")
        r1b0, s0, f0 = phase_a(0)
        state = {}

        def hook_a1():
            f0()
            state["bc10"] = ln_finalize(*s0)

        def a1_block_cb(m):
            normalize1_chunk(r1b0, h8_0, state["bc10"], m)

        r1b1, s1, f1 = phase_a(1, hook=hook_a1, block_cb=a1_block_cb)

        def hook_b0():
            f1()
            bc11 = ln_finalize(*s1)
            normalize1(r1b1, h8_1, bc11)

        v0, s20, f20 = phase_b(0, r1b0, h8_0, hook=hook_b0)

        def hook_b1():
            f20()
            bc20 = ln_finalize(*s20)
            phase_c_out(0, v0, bc20, tail=False)

        v1, s21, f21 = phase_b(1, r1b1, h8_1, hook=hook_b1)
        f21()
        bc21 = ln_finalize(*s21)
        phase_c_out(1, v1, bc21, tail=True)

    nc.compile()
    return nc


def _get_nc():
    global _CACHED_NC
    if _CACHED_NC is None:
        _CACHED_NC = _build()
    return _CACHED_NC


def _prep_inputs(x, w_kv, b_kv, w_fc, b_fc, w_mproj, b_mproj,
                 ln1_g, ln1_b, ln2_g, ln2_b):
    """Host-side shard + retile. Returns per-core input maps."""
    bf = ml_dtypes.bfloat16
    x_flat = np.ascontiguousarray(np.asarray(x, dtype=np.float32).reshape(B * S, E))

    # weights: [in, out] -> [m, p, o, c] tiles, in = o*128+p, out = m*128+c
    def retile(w, io, oo):
        w = np.asarray(w, dtype=np.float32).reshape(io, P, oo, P)
        return np.ascontiguousarray(w.transpose(2, 1, 0, 3))

    wkv_t = retile(w_kv, EO, EO).astype(bf)
    wfc_t = retile(w_fc, EO, FO)     # [FO, P, EO, P] f32
    wmp_t = retile(w_mproj, FO, EO)  # [EO, P, FO, P] f32

    # bf16 part pre-scaled to the shared fp8 product scale; fp8 part
    # quantized at weight scale SW.
    wfc_bf = (wfc_t[:, :, :BFFC, :] * FC_SCALE).astype(bf)
    wmp_bf = (wmp_t[:, :, :BFMP, :] * MP_SCALE).astype(bf)

    def f8pairs(wslice, nch):
        # [oo, P, nch, P] -> [oo, P, nch//2, 2, P] fp8 at scale SW
        if nch == 0:
            oo = wslice.shape[0]
            return np.zeros((oo, P, 1, 2, P), E4NP)
        q = (wslice * SW).astype(E4NP)
        oo = q.shape[0]
        return np.ascontiguousarray(q.reshape(oo, P, nch // 2, 2, P))

    wmp_f8 = f8pairs(wmp_t[:, :, BFMP:, :], F8MP)
    wfc_f8 = f8pairs(wfc_t[:, :, BFFC:, :], F8FC)

    def p2d(v):  # [n*P] -> [P, n] with chunk o in column o
        v = np.asarray(v, np.float32)
        return np.ascontiguousarray(v.reshape(-1, P).T)

    g1c = p2d(ln1_g)
    b1c = p2d(ln1_b)
    shared = {
        "wkv": wkv_t, "wfc": wfc_bf, "wmp": wmp_bf,
        "wmp8": wmp_f8, "wfc8": wfc_f8,
        "bkv": p2d(b_kv), "bfc": p2d(b_fc), "bmp": p2d(b_mproj),
        "g1": g1c, "b1": b1c,
        "g1s": g1c * SA_H, "b1s": b1c * SA_H,
        "g2": p2d(ln2_g), "b2": p2d(ln2_b),
    }
    in_maps = []
    for c in range(NCORES):
        xT = np.ascontiguousarray(x_flat[c * TOK:(c + 1) * TOK].T)  # [E, TOK] f32
        in_maps.append({"xf": xT, "xb": xT.astype(bf), **shared})
    return in_maps


def _run(inputs, trace=False):
    nc = _get_nc()
    in_maps = _prep_inputs(
        inputs["x"], inputs["w_kv"], inputs["b_kv"], inputs["w_fc"],
        inputs["b_fc"], inputs["w_mproj"], inputs["b_mproj"],
        inputs["ln1_g"], inputs["ln1_b"], inputs["ln2_g"], inputs["ln2_b"])
    res = run_bass_kernel_spmd(nc, in_maps, core_ids=list(range(NCORES)),
                               trace=trace)
    outs = [np.asarray(res.results[c]["out"], dtype=np.float32).T
            for c in range(NCORES)]
    full = np.concatenate(outs, axis=0).reshape(B, S, E)
    return full, res


def kernel(**inputs) -> np.ndarray:
    out, _ = _run(inputs, trace=False)
    return out


# revision 8
# speedup vs baseline: 1.1369x; 1.0229x over previous
"""Trainium2 Bass kernel for nn_Block_68719476955 (dense transformer block).

Math: with H=1 the attention softmax is over a singleton axis, so
attn_prob == 1.0 exactly and the whole attention reduces to
x @ w_kv + b_kv (w_attn / b_attn / mask do not affect the output).

The block computed per token row x_t (E=2048):
    t  = x @ w_kv + b_kv
    h  = LN(x + t) * g1 + b1
    u  = gelu(h @ w_fc + b_fc)          # exact gelu
    v  = u @ w_mproj + b_mproj
    out= LN(v + x) * g2 + b2

Distribution: pure data-parallel over the 8192 tokens across 8 cores
(1024 tokens/core), full weights on every core, no collectives.

Device layout: feature-major ("transposed") activations [E, tokens] so
every matmul runs with the weight block as the stationary operand
(lhsT = W[in,out] chunk, rhs = activation [in, tok]) and the output
lands feature-major again — zero on-device transposes. LayerNorm
reductions (over features = partitions) run on the TensorEngine as
ones-vector matmuls (chunk tiles pre-summed in quads on DVE), one
chunk behind the producing matmuls; per-token stats come back across
partitions via gpsimd.partition_broadcast.

Precision: bf16 matmul operands with fp32 PSUM accumulation, except a
slice of the mproj contraction (F8MP of 64 hidden chunks) and of the
fc contraction (F8FC of 16 h chunks) which run as fp8-e4m3 DoubleRow
pair-matmuls (2x MAC rate). All products within one PSUM group share
one power-of-2 scale: the fp8 path quantizes act*sa / weight*64, the
bf16 path pre-scales its weights by sa*64 on the host, and the
eviction divides by sa*64. Measured end-to-end rel err ~1.5e-2 vs the
2e-2 gate (bf16-only is ~3e-3; each fp8 chunk adds sqrt(f)-scaled
quantization noise).
"""

import numpy as np
import ml_dtypes
from contextlib import ExitStack

import concourse.bacc as bacc
import concourse.mybir as mybir
import concourse.tile as tile
from concourse.bass_utils import run_bass_kernel_spmd

P = 128
B, S, E = 4, 2048, 2048
H4 = 4 * E                 # 8192 mlp hidden
NCORES = 8
TOK = (B * S) // NCORES    # 1024 tokens per core
T = 512                    # token tile (2 per core)
NT = TOK // T
EO = E // P                # 16
FO = H4 // P               # 64
LN_EPS = 1e-5

# fp8 fractions: last F8MP of the 64 mproj k-chunks and last F8FC of the
# 16 fc k-chunks run as DoubleRow fp8 pairs (must be even).
F8MP = 16
F8FC = 2
BFMP = FO - F8MP
BFFC = EO - F8FC
SA_H = 8.0                 # h quantization scale for fc fp8 path
SW = 64.0                  # weight quantization scale for fp8 paths
MP_SCALE = SW              # mproj products: u(x1) * w(x64)
FC_SCALE = (SA_H * SW) if F8FC else 1.0

F32 = mybir.dt.float32
BF16 = mybir.dt.bfloat16
F8 = mybir.dt.float8e4
DRM = mybir.MatmulPerfMode.DoubleRow
AF = mybir.ActivationFunctionType
ALU = mybir.AluOpType
E4NP = ml_dtypes.float8_e4m3

_CACHED_NC = None


def _build():
    nc = bacc.Bacc(None, target_bir_lowering=False)

    xf_d = nc.dram_tensor("xf", [E, TOK], F32, kind="ExternalInput")
    xb_d = nc.dram_tensor("xb", [E, TOK], BF16, kind="ExternalInput")
    wkv_d = nc.dram_tensor("wkv", [EO, P, EO, P], BF16, kind="ExternalInput")
    wfc_d = nc.dram_tensor("wfc", [FO, P, BFFC, P], BF16, kind="ExternalInput")
    wmp_d = nc.dram_tensor("wmp", [EO, P, BFMP, P], BF16, kind="ExternalInput")
    wmp8_d = nc.dram_tensor("wmp8", [EO, P, max(F8MP // 2, 1), 2, P], F8,
                            kind="ExternalInput")
    wfc8_d = nc.dram_tensor("wfc8", [FO, P, max(F8FC // 2, 1), 2, P], F8,
                            kind="ExternalInput")
    bkv_d = nc.dram_tensor("bkv", [P, EO], F32, kind="ExternalInput")
    bfc_d = nc.dram_tensor("bfc", [P, FO], F32, kind="ExternalInput")
    bmp_d = nc.dram_tensor("bmp", [P, EO], F32, kind="ExternalInput")
    g1_d = nc.dram_tensor("g1", [P, EO], F32, kind="ExternalInput")
    b1_d = nc.dram_tensor("b1", [P, EO], F32, kind="ExternalInput")
    g1s_d = nc.dram_tensor("g1s", [P, EO], F32, kind="ExternalInput")
    b1s_d = nc.dram_tensor("b1s", [P, EO], F32, kind="ExternalInput")
    g2_d = nc.dram_tensor("g2", [P, EO], F32, kind="ExternalInput")
    b2_d = nc.dram_tensor("b2", [P, EO], F32, kind="ExternalInput")
    out_d = nc.dram_tensor("out", [E, TOK], F32, kind="ExternalOutput")

    with tile.TileContext(nc) as tc, ExitStack() as ctx:
        consts = ctx.enter_context(tc.tile_pool(name="consts", bufs=1))
        xbp = ctx.enter_context(tc.tile_pool(name="xbp", bufs=1))
        wp = ctx.enter_context(tc.tile_pool(name="wp", bufs=3))
        w8p = ctx.enter_context(tc.tile_pool(name="w8p", bufs=2))
        xcp = ctx.enter_context(tc.tile_pool(name="xcp", bufs=2))
        rbp = ctx.enter_context(tc.tile_pool(name="rbp", bufs=2))
        h8p = ctx.enter_context(tc.tile_pool(name="h8p", bufs=2))
        up = ctx.enter_context(tc.tile_pool(name="up", bufs=1))
        vp = ctx.enter_context(tc.tile_pool(name="vp", bufs=1))
        tmp = ctx.enter_context(tc.tile_pool(name="tmp", bufs=4))
        sqp = ctx.enter_context(tc.tile_pool(name="sqp", bufs=8))
        stp = ctx.enter_context(tc.tile_pool(name="stp", bufs=1))
        bcp = ctx.enter_context(tc.tile_pool(name="bcp", bufs=2))
        psmm = ctx.enter_context(tc.tile_pool(name="psmm", bufs=4, space="PSUM"))
        psst = ctx.enter_context(tc.tile_pool(name="psst", bufs=2, space="PSUM"))

        # x in bf16, one tile per 128-feature chunk so the kv matmuls can
        # start as soon as their chunk lands (16 parallel DMAs, issued first).
        xbs = []
        for k in range(EO):
            xk = xbp.tile([P, TOK], BF16, tag=f"xb{k}")
            eng = nc.gpsimd if k % 2 == 0 else nc.scalar
            eng.dma_start(xk[:], xb_d[k * P:(k + 1) * P, :])
            xbs.append(xk)

        # --- constants (gpsimd queue keeps Sync free for the weight stream) ---
        bkv_t = consts.tile([P, EO], F32)
        nc.gpsimd.dma_start(bkv_t[:], bkv_d[:, :])
        bfc_t = consts.tile([P, FO], F32)
        nc.gpsimd.dma_start(bfc_t[:], bfc_d[:, :])
        bmp_t = consts.tile([P, EO], F32)
        nc.gpsimd.dma_start(bmp_t[:], bmp_d[:, :])
        g1_t = consts.tile([P, EO], F32)
        nc.gpsimd.dma_start(g1_t[:], g1_d[:, :])
        b1_t = consts.tile([P, EO], F32)
        nc.gpsimd.dma_start(b1_t[:], b1_d[:, :])
        g1s_t = consts.tile([P, EO], F32)
        nc.gpsimd.dma_start(g1s_t[:], g1s_d[:, :])
        b1s_t = consts.tile([P, EO], F32)
        nc.gpsimd.dma_start(b1s_t[:], b1s_d[:, :])
        g2_t = consts.tile([P, EO], F32)
        nc.gpsimd.dma_start(g2_t[:], g2_d[:, :])
        b2_t = consts.tile([P, EO], F32)
        nc.gpsimd.dma_start(b2_t[:], b2_d[:, :])
        ones_col = consts.tile([P, 1], BF16)
        nc.vector.memset(ones_col[:], 1.0)
        eps_t = consts.tile([1, 1], F32)
        nc.vector.memset(eps_t[:], LN_EPS)

        # warm the PE (HAM clock gate) while input DMAs are in flight
        warm_rhs = consts.tile([P, T], BF16)
        nc.vector.memset(warm_rhs[:], 1.0)
        warm_ps = psst.tile([1, T], F32, tag="pss")
        for _ in range(16):
            nc.tensor.matmul(warm_ps[:], lhsT=ones_col[:], rhs=warm_rhs[:],
                             start=True, stop=True)

        def tsl(t):
            return slice(t * T, (t + 1) * T)

        def ln_finalize(ps_sum, ps_sq):
            """per-token mean/var from accumulated sums -> [P,2,T] bcast
            (slot 0 = mean*rstd, slot 1 = rstd). PE-free."""
            st = stp.tile([1, 3, T], F32, tag="st")
            nc.vector.tensor_scalar_mul(st[:, 0, :], ps_sum[:], 1.0 / E)   # mean
            nc.vector.tensor_scalar_mul(st[:, 1, :], ps_sq[:], 1.0 / E)    # E[x^2]
            nc.vector.tensor_mul(out=st[:, 2, :], in0=st[:, 0, :], in1=st[:, 0, :])
            nc.vector.tensor_tensor(st[:, 1, :], st[:, 1, :], st[:, 2, :],
                                    ALU.subtract)                          # var
            nc.scalar.activation(st[:, 2, :], st[:, 1, :], AF.Sqrt,
                                 bias=eps_t[:], scale=1.0)                 # sqrt(var+eps)
            nc.vector.reciprocal(out=st[:, 2, :], in_=st[:, 2, :])         # rstd
            nc.vector.tensor_mul(out=st[:, 1, :], in0=st[:, 0, :], in1=st[:, 2, :])
            # slots: [1]=mean*rstd, [2]=rstd -> bcast adjacent pair
            bc = bcp.tile([P, 2, T], F32, tag="bc")
            nc.gpsimd.partition_broadcast(bc[:], st[:, 1:3, :])
            return bc

        # ---------- phase A: kv matmul + residual + LN1 stats ----------
        def phase_a(t, hook=None, block_cb=None):
            r1b = rbp.tile([P, EO, T], BF16, tag="rb")
            ps_sum = psst.tile([1, T], F32, tag="pss")
            ps_sq = psst.tile([1, T], F32, tag="psq")
            pending = []  # software-pipeline the stats MMs behind the k-loops

            def emit_stats(qi):
                # quad-sum chunks 4qi..4qi+3 on DVE (in place), then one
                # ones-MM per quad quantity
                m0 = 4 * qi
                pr = sqp.tile([P, T], BF16, tag="sq")
                nc.vector.tensor_add(out=pr[:], in0=r1b[:, m0, :],
                                     in1=r1b[:, m0 + 1, :])
                nc.vector.tensor_tensor(pr[:], pr[:], r1b[:, m0 + 2, :], ALU.add)
                nc.vector.tensor_tensor(pr[:], pr[:], r1b[:, m0 + 3, :], ALU.add)
                sq0, sq1, sq2, sq3 = (pending.pop(0) for _ in range(4))
                pq = sqp.tile([P, T], BF16, tag="sq")
                nc.vector.tensor_add(out=pq[:], in0=sq0[:], in1=sq1[:])
                nc.vector.tensor_tensor(pq[:], pq[:], sq2[:], ALU.add)
                nc.vector.tensor_tensor(pq[:], pq[:], sq3[:], ALU.add)
                nc.tensor.matmul(ps_sum[:], lhsT=ones_col[:], rhs=pr[:],
                                 start=(qi == 0), stop=(qi == EO // 4 - 1))
                nc.tensor.matmul(ps_sq[:], lhsT=ones_col[:], rhs=pq[:],
                                 start=(qi == 0), stop=(qi == EO // 4 - 1))

            for m in range(EO):
                wt = wp.tile([P, EO, P], BF16, tag="w")
                nc.sync.dma_start(wt[:], wkv_d[m])
                ps = psmm.tile([P, T], F32, tag="ps")
                for k in range(EO):
                    nc.tensor.matmul(ps[:], lhsT=wt[:, k, :],
                                     rhs=xbs[k][:, tsl(t)],
                                     start=(k == 0), stop=(k == EO - 1))
                if m == 0 and hook is not None:
                    hook()  # previous phase's deferred stats MMs
                xc = xcp.tile([P, T], F32, tag="xc")
                nc.sync.dma_start(xc[:], xf_d[m * P:(m + 1) * P, tsl(t)])
                t1 = tmp.tile([P, T], F32, tag="t1")
                nc.scalar.activation(t1[:], ps[:], AF.Identity,
                                     bias=bkv_t[:, m:m + 1], scale=1.0)
                nc.vector.tensor_add(out=r1b[:, m, :], in0=t1[:], in1=xc[:])
                sq = sqp.tile([P, T], BF16, tag="sq")
                nc.vector.tensor_mul(out=sq[:], in0=r1b[:, m, :], in1=r1b[:, m, :])
                pending.append(sq)
                if m % 4 == 3 and m < EO - 1:
                    emit_stats(m // 4)
                if block_cb is not None:
                    block_cb(m)

            def finish():
                emit_stats(3)
            return r1b, (ps_sum, ps_sq), finish

        def normalize1_chunk(r1b, h8, bc, m):
            # in place: h overwrites r1b (WAR on the stats MMs is tracked);
            # the last F8FC chunks also get an fp8 copy at scale SA_H.
            t1 = tmp.tile([P, T], F32, tag="t1")
            nc.vector.tensor_mul(out=t1[:], in0=r1b[:, m, :], in1=bc[:, 1, :])
            nc.vector.tensor_tensor(t1[:], t1[:], bc[:, 0, :], ALU.subtract)
            if m >= BFFC:
                nc.scalar.activation(h8[:, m - BFFC, :], t1[:], AF.Identity,
                                     bias=b1s_t[:, m:m + 1],
                                     scale=g1s_t[:, m:m + 1])
            if m < BFFC:
                nc.scalar.activation(r1b[:, m, :], t1[:], AF.Identity,
                                     bias=b1_t[:, m:m + 1],
                                     scale=g1_t[:, m:m + 1])

        def normalize1(r1b, h8, bc):
            for m in range(EO):
                normalize1_chunk(r1b, h8, bc, m)

        # ---------- phase B: mlp; LN2 stats fused into last-chunk evicts ----
        def phase_b(t, h, h8, hook=None):
            v2f = vp.tile([P, EO, T], F32, tag="v")     # r2 = v + b_mproj + x
            ps_sum = psst.tile([1, T], F32, tag="pss")
            ps_sq = psst.tile([1, T], F32, tag="psq")
            pending = []

            def emit_stats2(qi):
                # r2 chunk sums read straight from v2f (f32) with bf16 out
                m0 = 4 * qi
                s0, s1, s2, s3 = (pending.pop(0) for _ in range(4))
                pr = sqp.tile([P, T], BF16, tag="sq")
                nc.vector.tensor_add(out=pr[:], in0=v2f[:, m0, :],
                                     in1=v2f[:, m0 + 1, :])
                nc.vector.tensor_tensor(pr[:], pr[:], v2f[:, m0 + 2, :], ALU.add)
                nc.vector.tensor_tensor(pr[:], pr[:], v2f[:, m0 + 3, :], ALU.add)
                pq = sqp.tile([P, T], BF16, tag="sq")
                nc.vector.tensor_add(out=pq[:], in0=s0[:], in1=s1[:])
                nc.vector.tensor_tensor(pq[:], pq[:], s2[:], ALU.add)
                nc.vector.tensor_tensor(pq[:], pq[:], s3[:], ALU.add)
                nc.tensor.matmul(ps_sum[:], lhsT=ones_col[:], rhs=pr[:],
                                 start=(qi == 0), stop=(qi == EO // 4 - 1))
                nc.tensor.matmul(ps_sq[:], lhsT=ones_col[:], rhs=pq[:],
                                 start=(qi == 0), stop=(qi == EO // 4 - 1))

            ub = up.tile([P, BFMP, T], BF16, tag="u")
            u8 = up.tile([P, max(F8MP, 1), T], F8, tag="u8")
            for ma in range(FO):
                wt = wp.tile([P, BFFC, P], BF16, tag="w")
                nc.sync.dma_start(wt[:], wfc_d[ma])
                if F8FC:
                    wt8 = w8p.tile([P, F8FC // 2, 2, P], F8, tag="w8")
                    nc.sync.dma_start(wt8[:], wfc8_d[ma])
                ps = psmm.tile([P, T], F32, tag="ps")
                for k in range(BFFC):
                    nc.tensor.matmul(ps[:], lhsT=wt[:, k, :], rhs=h[:, k, :],
                                     start=(k == 0), stop=(F8FC == 0 and k == BFFC - 1))
                for j in range(F8FC // 2):
                    nc.tensor.matmul(ps[:], lhsT=wt8[:, j, :, :],
                                     rhs=h8[:, 2 * j:2 * j + 2, :],
                                     start=False, stop=(j == F8FC // 2 - 1),
                                     perf_mode=DRM)
                if ma == 0 and hook is not None:
                    hook()  # previous phase's deferred stats + LN2 output
                if ma < BFMP:
                    nc.scalar.activation(ub[:, ma, :], ps[:], AF.Gelu,
                                         bias=bfc_t[:, ma:ma + 1],
                                         scale=1.0 / FC_SCALE)
                else:
                    nc.scalar.activation(u8[:, ma - BFMP, :], ps[:], AF.Gelu,
                                         bias=bfc_t[:, ma:ma + 1],
                                         scale=1.0 / FC_SCALE)
            for mo in range(EO):
                ps = psmm.tile([P, T], F32, tag="ps")
                done = 0
                for nk in (16,) * (BFMP // 16) + ((BFMP % 16,) if BFMP % 16 else ()):
                    wt = wp.tile([P, nk, P], BF16, tag="w")
                    nc.sync.dma_start(wt[:], wmp_d[mo][:, done:done + nk, :])
                    for k in range(nk):
                        nc.tensor.matmul(ps[:], lhsT=wt[:, k, :],
                                         rhs=ub[:, done + k, :],
                                         start=(done + k == 0), stop=False)
                    done += nk
                wt8 = w8p.tile([P, F8MP // 2, 2, P], F8, tag="w8")
                nc.sync.dma_start(wt8[:], wmp8_d[mo])
                for j in range(F8MP // 2):
                    nc.tensor.matmul(ps[:], lhsT=wt8[:, j, :, :],
                                     rhs=u8[:, 2 * j:2 * j + 2, :],
                                     start=False, stop=(j == F8MP // 2 - 1),
                                     perf_mode=DRM)
                xc = xcp.tile([P, T], F32, tag="xc")
                nc.gpsimd.dma_start(xc[:], xf_d[mo * P:(mo + 1) * P, tsl(t)])
                nc.scalar.activation(v2f[:, mo, :], ps[:], AF.Identity,
                                     bias=bmp_t[:, mo:mo + 1],
                                     scale=1.0 / MP_SCALE)
                nc.vector.tensor_add(out=v2f[:, mo, :], in0=v2f[:, mo, :],
                                     in1=xc[:])
                sq = sqp.tile([P, T], BF16, tag="sq")
                nc.vector.tensor_mul(out=sq[:], in0=v2f[:, mo, :],
                                     in1=v2f[:, mo, :])
                pending.append(sq)
                if mo % 4 == 3 and mo < EO - 1:
                    emit_stats2(mo // 4)

            def finish():
                emit_stats2(3)
            return v2f, (ps_sum, ps_sq), finish

        # ---------- phase C: final normalize + output ----------
        def phase_c_out(t, v2f, bc, tail):
            dma_eng = nc.sync if tail else nc.gpsimd
            for m in range(EO):
                t1 = tmp.tile([P, T], F32, tag="t1")
                nc.vector.tensor_mul(out=t1[:], in0=v2f[:, m, :], in1=bc[:, 1, :])
                nc.vector.tensor_tensor(t1[:], t1[:], bc[:, 0, :], ALU.subtract)
                if tail:
                    nc.scalar.activation(t1[:], t1[:], AF.Identity,
                                         bias=b2_t[:, m:m + 1],
                                         scale=g2_t[:, m:m + 1])
                else:
                    nc.vector.tensor_scalar(t1[:], t1[:], g2_t[:, m:m + 1],
                                            b2_t[:, m:m + 1], ALU.mult, ALU.add)
                dma_eng.dma_start(out_d[m * P:(m + 1) * P, tsl(t)], t1[:])

        # Emission order interleaves the two token tiles so the PE never
        # waits on DVE normalize chains or LN finalize chains; each phase's
        # final stats MMs are deferred into the next phase's first block.
        h8_0 = h8p.tile([P, max(F8FC, 1), T], F8, tag="h8")
        h8_1 = h8p.tile([P, max(F8FC, 1), T], F8, tag="h8")
        r1b0, s0, f0 = phase_a(0)
        state = {}

        def hook_a1():
            f0()
            state["bc10"] = ln_finalize(*s0)

        def a1_block_cb(m):
            normalize1_chunk(r1b0, h8_0, state["bc10"], m)

        r1b1, s1, f1 = phase_a(1, hook=hook_a1, block_cb=a1_block_cb)

        def hook_b0():
            f1()
            bc11 = ln_finalize(*s1)
            normalize1(r1b1, h8_1, bc11)

        v0, s20, f20 = phase_b(0, r1b0, h8_0, hook=hook_b0)

        def hook_b1():
            f20()
            bc20 = ln_finalize(*s20)
            phase_c_out(0, v0, bc20, tail=False)

        v1, s21, f21 = phase_b(1, r1b1, h8_1, hook=hook_b1)
        f21()
        bc21 = ln_finalize(*s21)
        phase_c_out(1, v1, bc21, tail=True)

    nc.compile()
    return nc


def _get_nc():
    global _CACHED_NC
    if _CACHED_NC is None:
        _CACHED_NC = _build()
    return _CACHED_NC


def _prep_inputs(x, w_kv, b_kv, w_fc, b_fc, w_mproj, b_mproj,
                 ln1_g, ln1_b, ln2_g, ln2_b):
    """Host-side shard + retile. Returns per-core input maps."""
    bf = ml_dtypes.bfloat16
    x_flat = np.ascontiguousarray(np.asarray(x, dtype=np.float32).reshape(B * S, E))

    # weights: [in, out] -> [m, p, o, c] tiles, in = o*128+p, out = m*128+c
    def retile(w, io, oo):
        w = np.asarray(w, dtype=np.float32).reshape(io, P, oo, P)
        return np.ascontiguousarray(w.transpose(2, 1, 0, 3))

    wkv_t = retile(w_kv, EO, EO).astype(bf)
    wfc_t = retile(w_fc, EO, FO)     # [FO, P, EO, P] f32
    wmp_t = retile(w_mproj, FO, EO)  # [EO, P, FO, P] f32

    # bf16 part pre-scaled to the shared fp8 product scale; fp8 part
    # quantized at weight scale SW.
    wfc_bf = (wfc_t[:, :, :BFFC, :] * FC_SCALE).astype(bf)
    wmp_bf = (wmp_t[:, :, :BFMP, :] * MP_SCALE).astype(bf)

    def f8pairs(wslice, nch):
        # [oo, P, nch, P] -> [oo, P, nch//2, 2, P] fp8 at scale SW
        if nch == 0:
            oo = wslice.shape[0]
            return np.zeros((oo, P, 1, 2, P), E4NP)
        q = (wslice * SW).astype(E4NP)
        oo = q.shape[0]
        return np.ascontiguousarray(q.reshape(oo, P, nch // 2, 2, P))

    wmp_f8 = f8pairs(wmp_t[:, :, BFMP:, :], F8MP)
    wfc_f8 = f8pairs(wfc_t[:, :, BFFC:, :], F8FC)

    def p2d(v):  # [n*P] -> [P, n] with chunk o in column o
        v = np.asarray(v, np.float32)
        return np.ascontiguousarray(v.reshape(-1, P).T)

    g1c = p2d(ln1_g)
    b1c = p2d(ln1_b)
    shared = {
        "wkv": wkv_t, "wfc": wfc_bf, "wmp": wmp_bf,
        "wmp8": wmp_f8, "wfc8": wfc_f8,
        "bkv": p2d(b_kv), "bfc": p2d(b_fc), "bmp": p2d(b_mproj),
        "g1": g1c, "b1": b1c,
        "g1s": g1c * SA_H, "b1s": b1c * SA_H,
        "g2": p2d(ln2_g), "b2": p2d(ln2_b),
    }
    in_maps = []
    for c in range(NCORES):
        xT = np.ascontiguousarray(x_flat[c * TOK:(c + 1) * TOK].T)  # [E, TOK] f32
        in_maps.append({"xf": xT, "xb": xT.astype(bf), **shared})
    return in_maps


def _run(inputs, trace=False):
    nc = _get_nc()
    in_maps = _prep_inputs(
        inputs["x"], inputs["w_kv"], inputs["b_kv"], inputs["w_fc"],
        inputs["b_fc"], inputs["w_mproj"], inputs["b_mproj"],
        inputs["ln1_g"], inputs["ln1_b"], inputs["ln2_g"], inputs["ln2_b"])
    res = run_bass_kernel_spmd(nc, in_maps, core_ids=list(range(NCORES)),
                               trace=trace)
    outs = [np.asarray(res.results[c]["out"], dtype=np.float32).T
            for c in range(NCORES)]
    full = np.concatenate(outs, axis=0).reshape(B, S, E)
    return full, res


def kernel(**inputs) -> np.ndarray:
    out, _ = _run(inputs, trace=False)
    return out


# revision 24
# speedup vs baseline: 1.1435x; 1.0059x over previous
"""Trainium2 Bass kernel for nn_Block_68719476955 (dense transformer block).

Math: with H=1 the attention softmax is over a singleton axis, so
attn_prob == 1.0 exactly and the whole attention reduces to
x @ w_kv + b_kv (w_attn / b_attn / mask do not affect the output).

The block computed per token row x_t (E=2048):
    t  = x @ w_kv + b_kv
    h  = LN(x + t) * g1 + b1
    u  = gelu(h @ w_fc + b_fc)          # exact gelu
    v  = u @ w_mproj + b_mproj
    out= LN(v + x) * g2 + b2

Distribution: pure data-parallel over the 8192 tokens across 8 cores
(1024 tokens/core), full weights on every core, no collectives.

Device layout: feature-major ("transposed") activations [E, tokens] so
every matmul runs with the weight block as the stationary operand
(lhsT = W[in,out] chunk, rhs = activation [in, tok]) and the output
lands feature-major again — zero on-device transposes. LayerNorm
reductions (over features = partitions) run on the TensorEngine as
ones-vector matmuls (chunk tiles pre-summed in quads on DVE), one
chunk behind the producing matmuls; per-token stats come back across
partitions via gpsimd.partition_broadcast.

Precision: bf16 matmul operands with fp32 PSUM accumulation, except a
slice of the mproj contraction (F8MP of 64 hidden chunks) and of the
fc contraction (F8FC of 16 h chunks) which run as fp8-e4m3 DoubleRow
pair-matmuls (2x MAC rate). All products within one PSUM group share
one power-of-2 scale: the fp8 path quantizes act*sa / weight*64, the
bf16 path pre-scales its weights by sa*64 on the host, and the
eviction divides by sa*64. Measured end-to-end rel err ~1.5e-2 vs the
2e-2 gate (bf16-only is ~3e-3; each fp8 chunk adds sqrt(f)-scaled
quantization noise).
"""

import numpy as np
import ml_dtypes
from contextlib import ExitStack

import concourse.bacc as bacc
import concourse.mybir as mybir
import concourse.tile as tile
from concourse.bass_utils import run_bass_kernel_spmd

P = 128
B, S, E = 4, 2048, 2048
H4 = 4 * E                 # 8192 mlp hidden
NCORES = 8
TOK = (B * S) // NCORES    # 1024 tokens per core
T = 512                    # token tile (2 per core)
NT = TOK // T
EO = E // P                # 16
FO = H4 // P               # 64
LN_EPS = 1e-5

# fp8 fractions: last F8MP of the 64 mproj k-chunks and last F8FC of the
# 16 fc k-chunks run as DoubleRow fp8 pairs (must be even).
F8MP = 16
F8FC = 2
BFMP = FO - F8MP
BFFC = EO - F8FC
SA_H = 8.0                 # h quantization scale for fc fp8 path
SW = 64.0                  # weight quantization scale for fp8 paths
WARM = 16                  # PE warm-up matmuls (p-state ramp + startup DMA)
MP_SCALE = SW              # mproj products: u(x1) * w(x64)
FC_SCALE = (SA_H * SW) if F8FC else 1.0

F32 = mybir.dt.float32
BF16 = mybir.dt.bfloat16
F8 = mybir.dt.float8e4
DRM = mybir.MatmulPerfMode.DoubleRow
AF = mybir.ActivationFunctionType
ALU = mybir.AluOpType
E4NP = ml_dtypes.float8_e4m3

_CACHED_NC = None


def _build():
    nc = bacc.Bacc(None, target_bir_lowering=False)

    xf_d = nc.dram_tensor("xf", [E, TOK], F32, kind="ExternalInput")
    xb_d = nc.dram_tensor("xb", [E, TOK], BF16, kind="ExternalInput")
    wkv_d = nc.dram_tensor("wkv", [EO, P, EO, P], BF16, kind="ExternalInput")
    wfc_d = nc.dram_tensor("wfc", [FO, P, BFFC, P], BF16, kind="ExternalInput")
    wmp_d = nc.dram_tensor("wmp", [EO, P, BFMP, P], BF16, kind="ExternalInput")
    wmp8_d = nc.dram_tensor("wmp8", [EO, P, max(F8MP // 2, 1), 2, P], F8,
                            kind="ExternalInput")
    wfc8_d = nc.dram_tensor("wfc8", [FO, P, max(F8FC // 2, 1), 2, P], F8,
                            kind="ExternalInput")
    bkv_d = nc.dram_tensor("bkv", [P, EO], F32, kind="ExternalInput")
    bfc_d = nc.dram_tensor("bfc", [P, FO], F32, kind="ExternalInput")
    bmp_d = nc.dram_tensor("bmp", [P, EO], F32, kind="ExternalInput")
    g1_d = nc.dram_tensor("g1", [P, EO], F32, kind="ExternalInput")
    b1_d = nc.dram_tensor("b1", [P, EO], F32, kind="ExternalInput")
    g1s_d = nc.dram_tensor("g1s", [P, EO], F32, kind="ExternalInput")
    b1s_d = nc.dram_tensor("b1s", [P, EO], F32, kind="ExternalInput")
    g2_d = nc.dram_tensor("g2", [P, EO], F32, kind="ExternalInput")
    b2_d = nc.dram_tensor("b2", [P, EO], F32, kind="ExternalInput")
    out_d = nc.dram_tensor("out", [E, TOK], F32, kind="ExternalOutput")

    with tile.TileContext(nc) as tc, ExitStack() as ctx:
        consts = ctx.enter_context(tc.tile_pool(name="consts", bufs=1))
        xbp = ctx.enter_context(tc.tile_pool(name="xbp", bufs=1))
        wp = ctx.enter_context(tc.tile_pool(name="wp", bufs=3))
        w8p = ctx.enter_context(tc.tile_pool(name="w8p", bufs=2))
        xcp = ctx.enter_context(tc.tile_pool(name="xcp", bufs=2))
        rbp = ctx.enter_context(tc.tile_pool(name="rbp", bufs=2))
        h8p = ctx.enter_context(tc.tile_pool(name="h8p", bufs=2))
        up = ctx.enter_context(tc.tile_pool(name="up", bufs=1))
        vp = ctx.enter_context(tc.tile_pool(name="vp", bufs=1))
        tmp = ctx.enter_context(tc.tile_pool(name="tmp", bufs=4))
        sqp = ctx.enter_context(tc.tile_pool(name="sqp", bufs=8))
        stp = ctx.enter_context(tc.tile_pool(name="stp", bufs=1))
        bcp = ctx.enter_context(tc.tile_pool(name="bcp", bufs=2))
        psmm = ctx.enter_context(tc.tile_pool(name="psmm", bufs=4, space="PSUM"))
        psst = ctx.enter_context(tc.tile_pool(name="psst", bufs=2, space="PSUM"))

        def tsl(t):
            return slice(t * T, (t + 1) * T)

        # x in bf16, one tile per 128-feature chunk so the kv matmuls can
        # start as soon as their chunk lands. Token-tile-0 halves are issued
        # first across three queues so phase_a(0) is gated on half the bytes.
        xbs = []
        engs = (nc.gpsimd, nc.scalar)
        for k in range(EO):
            xk = xbp.tile([P, TOK], BF16, tag=f"xb{k}")
            xbs.append(xk)
        for half in range(NT):
            for k in range(EO):
                engs[k % 2].dma_start(xbs[k][:, tsl(half)],
                                      xb_d[k * P:(k + 1) * P, tsl(half)])

        # --- constants (gpsimd queue keeps Sync free for the weight stream) ---
        bkv_t = consts.tile([P, EO], F32)
        nc.gpsimd.dma_start(bkv_t[:], bkv_d[:, :])
        bfc_t = consts.tile([P, FO], F32)
        nc.gpsimd.dma_start(bfc_t[:], bfc_d[:, :])
        bmp_t = consts.tile([P, EO], F32)
        nc.gpsimd.dma_start(bmp_t[:], bmp_d[:, :])
        g1_t = consts.tile([P, EO], F32)
        nc.gpsimd.dma_start(g1_t[:], g1_d[:, :])
        b1_t = consts.tile([P, EO], F32)
        nc.gpsimd.dma_start(b1_t[:], b1_d[:, :])
        g1s_t = consts.tile([P, EO], F32)
        nc.gpsimd.dma_start(g1s_t[:], g1s_d[:, :])
        b1s_t = consts.tile([P, EO], F32)
        nc.gpsimd.dma_start(b1s_t[:], b1s_d[:, :])
        g2_t = consts.tile([P, EO], F32)
        nc.gpsimd.dma_start(g2_t[:], g2_d[:, :])
        b2_t = consts.tile([P, EO], F32)
        nc.gpsimd.dma_start(b2_t[:], b2_d[:, :])
        ones_col = consts.tile([P, 1], BF16)
        nc.vector.memset(ones_col[:], 1.0)
        eps_t = consts.tile([1, 1], F32)
        nc.vector.memset(eps_t[:], LN_EPS)

        # warm the PE (HAM clock gate) while input DMAs are in flight
        warm_rhs = consts.tile([P, T], BF16)
        nc.vector.memset(warm_rhs[:], 1.0)
        warm_ps = psst.tile([1, T], F32, tag="pss")
        for _ in range(WARM):
            nc.tensor.matmul(warm_ps[:], lhsT=ones_col[:], rhs=warm_rhs[:],
                             start=True, stop=True)

        def ln_finalize(ps_sum, ps_sq):
            """per-token mean/var from accumulated sums -> [P,2,T] bcast
            (slot 0 = mean*rstd, slot 1 = rstd). PE-free."""
            st = stp.tile([1, 3, T], F32, tag="st")
            nc.vector.tensor_scalar_mul(st[:, 0, :], ps_sum[:], 1.0 / E)   # mean
            nc.vector.tensor_scalar_mul(st[:, 1, :], ps_sq[:], 1.0 / E)    # E[x^2]
            nc.vector.tensor_mul(out=st[:, 2, :], in0=st[:, 0, :], in1=st[:, 0, :])
            nc.vector.tensor_tensor(st[:, 1, :], st[:, 1, :], st[:, 2, :],
                                    ALU.subtract)                          # var
            nc.scalar.activation(st[:, 2, :], st[:, 1, :], AF.Sqrt,
                                 bias=eps_t[:], scale=1.0)                 # sqrt(var+eps)
            nc.vector.reciprocal(out=st[:, 2, :], in_=st[:, 2, :])         # rstd
            nc.vector.tensor_mul(out=st[:, 1, :], in0=st[:, 0, :], in1=st[:, 2, :])
            # slots: [1]=mean*rstd, [2]=rstd -> bcast adjacent pair
            bc = bcp.tile([P, 2, T], F32, tag="bc")
            nc.gpsimd.partition_broadcast(bc[:], st[:, 1:3, :])
            return bc

        # ---------- phase A: kv matmul + residual + LN1 stats ----------
        def phase_a(t, hook=None, block_cb=None):
            r1b = rbp.tile([P, EO, T], BF16, tag="rb")
            ps_sum = psst.tile([1, T], F32, tag="pss")
            ps_sq = psst.tile([1, T], F32, tag="psq")
            pending = []  # software-pipeline the stats MMs behind the k-loops

            presums = {}

            def emit_dve(qi):
                # quad-sum chunks 4qi..4qi+3 on DVE (in place)
                m0 = 4 * qi
                pr = sqp.tile([P, T], BF16, tag="sq")
                nc.vector.tensor_add(out=pr[:], in0=r1b[:, m0, :],
                                     in1=r1b[:, m0 + 1, :])
                nc.vector.tensor_tensor(pr[:], pr[:], r1b[:, m0 + 2, :], ALU.add)
                nc.vector.tensor_tensor(pr[:], pr[:], r1b[:, m0 + 3, :], ALU.add)
                sq0, sq1, sq2, sq3 = (pending.pop(0) for _ in range(4))
                pq = sqp.tile([P, T], BF16, tag="sq")
                nc.vector.tensor_add(out=pq[:], in0=sq0[:], in1=sq1[:])
                nc.vector.tensor_tensor(pq[:], pq[:], sq2[:], ALU.add)
                nc.vector.tensor_tensor(pq[:], pq[:], sq3[:], ALU.add)
                presums[qi] = (pr, pq)

            def emit_mm(qi):
                pr, pq = presums.pop(qi)
                nc.tensor.matmul(ps_sum[:], lhsT=ones_col[:], rhs=pr[:],
                                 start=(qi == 0), stop=(qi == EO // 4 - 1))
                nc.tensor.matmul(ps_sq[:], lhsT=ones_col[:], rhs=pq[:],
                                 start=(qi == 0), stop=(qi == EO // 4 - 1))

            for m in range(EO):
                wt = wp.tile([P, EO, P], BF16, tag="w")
                if t == 0 and m < 2:
                    # startup: quarter-DMAs so the first matmuls aren't gated
                    # on the full 512KB weight transfer
                    for qq in range(4):
                        nc.sync.dma_start(wt[:, 4 * qq:4 * qq + 4, :],
                                          wkv_d[m][:, 4 * qq:4 * qq + 4, :])
                else:
                    nc.sync.dma_start(wt[:], wkv_d[m])
                ps = psmm.tile([P, T], F32, tag="ps")
                for k in range(EO):
                    nc.tensor.matmul(ps[:], lhsT=wt[:, k, :],
                                     rhs=xbs[k][:, tsl(t)],
                                     start=(k == 0), stop=(k == EO - 1))
                if m == 0 and hook is not None:
                    hook()  # previous phase's deferred stats MMs
                xc = xcp.tile([P, T], F32, tag="xc")
                nc.sync.dma_start(xc[:], xf_d[m * P:(m + 1) * P, tsl(t)])
                t1 = tmp.tile([P, T], F32, tag="t1")
                nc.scalar.activation(t1[:], ps[:], AF.Identity,
                                     bias=bkv_t[:, m:m + 1], scale=1.0)
                nc.vector.tensor_add(out=r1b[:, m, :], in0=t1[:], in1=xc[:])
                sq = sqp.tile([P, T], BF16, tag="sq")
                nc.vector.tensor_mul(out=sq[:], in0=r1b[:, m, :], in1=r1b[:, m, :])
                pending.append(sq)
                # DVE presums as soon as the quad's data lands; the ones-MMs
                # two blocks later so the PE never waits on the DVE chain.
                if m % 4 == 3 and m < EO - 1:
                    emit_dve(m // 4)
                if m % 4 == 1 and m > 4:
                    emit_mm(m // 4 - 1)
                if block_cb is not None:
                    block_cb(m)

            def finish():
                emit_dve(3)
                emit_mm(3)
            return r1b, (ps_sum, ps_sq), finish

        def normalize1_chunk(r1b, h8, bc, m):
            # in place: h overwrites r1b (WAR on the stats MMs is tracked);
            # the last F8FC chunks also get an fp8 copy at scale SA_H.
            t1 = tmp.tile([P, T], F32, tag="t1")
            nc.vector.tensor_mul(out=t1[:], in0=r1b[:, m, :], in1=bc[:, 1, :])
            nc.vector.tensor_tensor(t1[:], t1[:], bc[:, 0, :], ALU.subtract)
            if m >= BFFC:
                nc.scalar.activation(h8[:, m - BFFC, :], t1[:], AF.Identity,
                                     bias=b1s_t[:, m:m + 1],
                                     scale=g1s_t[:, m:m + 1])
            if m < BFFC:
                nc.scalar.activation(r1b[:, m, :], t1[:], AF.Identity,
                                     bias=b1_t[:, m:m + 1],
                                     scale=g1_t[:, m:m + 1])

        def normalize1(r1b, h8, bc):
            for m in range(EO):
                normalize1_chunk(r1b, h8, bc, m)

        # ---------- phase B: mlp; LN2 stats fused into last-chunk evicts ----
        def phase_b(t, h, h8, hook=None):
            v2f = vp.tile([P, EO, T], F32, tag="v")     # r2 = v + b_mproj + x
            ps_sum = psst.tile([1, T], F32, tag="pss")
            ps_sq = psst.tile([1, T], F32, tag="psq")
            pending = []

            presums2 = {}

            def emit_dve2(qi):
                # r2 chunk sums read straight from v2f (f32) with bf16 out
                m0 = 4 * qi
                s0, s1, s2, s3 = (pending.pop(0) for _ in range(4))
                pr = sqp.tile([P, T], BF16, tag="sq")
                nc.vector.tensor_add(out=pr[:], in0=v2f[:, m0, :],
                                     in1=v2f[:, m0 + 1, :])
                nc.vector.tensor_tensor(pr[:], pr[:], v2f[:, m0 + 2, :], ALU.add)
                nc.vector.tensor_tensor(pr[:], pr[:], v2f[:, m0 + 3, :], ALU.add)
                pq = sqp.tile([P, T], BF16, tag="sq")
                nc.vector.tensor_add(out=pq[:], in0=s0[:], in1=s1[:])
                nc.vector.tensor_tensor(pq[:], pq[:], s2[:], ALU.add)
                nc.vector.tensor_tensor(pq[:], pq[:], s3[:], ALU.add)
                presums2[qi] = (pr, pq)

            def emit_mm2(qi):
                pr, pq = presums2.pop(qi)
                nc.tensor.matmul(ps_sum[:], lhsT=ones_col[:], rhs=pr[:],
                                 start=(qi == 0), stop=(qi == EO // 4 - 1))
                nc.tensor.matmul(ps_sq[:], lhsT=ones_col[:], rhs=pq[:],
                                 start=(qi == 0), stop=(qi == EO // 4 - 1))

            ub = up.tile([P, BFMP, T], BF16, tag="u")
            u8 = up.tile([P, max(F8MP, 1), T], F8, tag="u8")
            for ma in range(FO):
                wt = wp.tile([P, BFFC, P], BF16, tag="w")
                nc.sync.dma_start(wt[:], wfc_d[ma])
                if F8FC:
                    wt8 = w8p.tile([P, F8FC // 2, 2, P], F8, tag="w8")
                    nc.sync.dma_start(wt8[:], wfc8_d[ma])
                ps = psmm.tile([P, T], F32, tag="ps")
                for k in range(BFFC):
                    nc.tensor.matmul(ps[:], lhsT=wt[:, k, :], rhs=h[:, k, :],
                                     start=(k == 0), stop=(F8FC == 0 and k == BFFC - 1))
                for j in range(F8FC // 2):
                    nc.tensor.matmul(ps[:], lhsT=wt8[:, j, :, :],
                                     rhs=h8[:, 2 * j:2 * j + 2, :],
                                     start=False, stop=(j == F8FC // 2 - 1),
                                     perf_mode=DRM)
                if ma == 0 and hook is not None:
                    hook()  # previous phase's deferred stats + LN2 output
                if ma < BFMP:
                    nc.scalar.activation(ub[:, ma, :], ps[:], AF.Gelu,
                                         bias=bfc_t[:, ma:ma + 1],
                                         scale=1.0 / FC_SCALE)
                else:
                    nc.scalar.activation(u8[:, ma - BFMP, :], ps[:], AF.Gelu,
                                         bias=bfc_t[:, ma:ma + 1],
                                         scale=1.0 / FC_SCALE)
            for mo in range(EO):
                ps = psmm.tile([P, T], F32, tag="ps")
                done = 0
                for nk in (16,) * (BFMP // 16) + ((BFMP % 16,) if BFMP % 16 else ()):
                    wt = wp.tile([P, nk, P], BF16, tag="w")
                    nc.sync.dma_start(wt[:], wmp_d[mo][:, done:done + nk, :])
                    for k in range(nk):
                        nc.tensor.matmul(ps[:], lhsT=wt[:, k, :],
                                         rhs=ub[:, done + k, :],
                                         start=(done + k == 0), stop=False)
                    done += nk
                wt8 = w8p.tile([P, F8MP // 2, 2, P], F8, tag="w8")
                nc.sync.dma_start(wt8[:], wmp8_d[mo])
                for j in range(F8MP // 2):
                    nc.tensor.matmul(ps[:], lhsT=wt8[:, j, :, :],
                                     rhs=u8[:, 2 * j:2 * j + 2, :],
                                     start=False, stop=(j == F8MP // 2 - 1),
                                     perf_mode=DRM)
                xc = xcp.tile([P, T], F32, tag="xc")
                nc.gpsimd.dma_start(xc[:], xf_d[mo * P:(mo + 1) * P, tsl(t)])
                nc.scalar.activation(v2f[:, mo, :], ps[:], AF.Identity,
                                     bias=bmp_t[:, mo:mo + 1],
                                     scale=1.0 / MP_SCALE)
                nc.vector.tensor_add(out=v2f[:, mo, :], in0=v2f[:, mo, :],
                                     in1=xc[:])
                sq = sqp.tile([P, T], BF16, tag="sq")
                nc.vector.tensor_mul(out=sq[:], in0=v2f[:, mo, :],
                                     in1=v2f[:, mo, :])
                pending.append(sq)
                if mo % 4 == 3 and mo < EO - 1:
                    emit_dve2(mo // 4)
                if mo % 4 == 1 and mo > 4:
                    emit_mm2(mo // 4 - 1)

            def finish():
                emit_dve2(3)
                emit_mm2(3)
            return v2f, (ps_sum, ps_sq), finish

        # ---------- phase C: final normalize + output ----------
        def phase_c_out(t, v2f, bc, tail):
            dma_eng = nc.sync if tail else nc.gpsimd
            for m in range(EO):
                # at the tail nothing overlaps the normalize chain; spread a
                # quarter of the chunks onto the otherwise-idle gpsimd
                veng = nc.gpsimd if (tail and m % 4 == 2) else nc.vector
                t1 = tmp.tile([P, T], F32, tag="t1")
                veng.tensor_mul(out=t1[:], in0=v2f[:, m, :], in1=bc[:, 1, :])
                veng.tensor_tensor(t1[:], t1[:], bc[:, 0, :], ALU.subtract)
                if tail:
                    nc.scalar.activation(t1[:], t1[:], AF.Identity,
                                         bias=b2_t[:, m:m + 1],
                                         scale=g2_t[:, m:m + 1])
                else:
                    nc.vector.tensor_scalar(t1[:], t1[:], g2_t[:, m:m + 1],
                                            b2_t[:, m:m + 1], ALU.mult, ALU.add)
                dma_eng.dma_start(out_d[m * P:(m + 1) * P, tsl(t)], t1[:])

        # Emission order interleaves the two token tiles so the PE never
        # waits on DVE normalize chains or LN finalize chains; each phase's
        # final stats MMs are deferred into the next phase's first block.
        h8_0 = h8p.tile([P, max(F8FC, 1), T], F8, tag="h8")
        h8_1 = h8p.tile([P, max(F8FC, 1), T], F8, tag="h8")
        r1b0, s0, f0 = phase_a(0)
        state = {}

        def hook_a1():
            f0()
            state["bc10"] = ln_finalize(*s0)

        def a1_block_cb(m):
            normalize1_chunk(r1b0, h8_0, state["bc10"], m)

        r1b1, s1, f1 = phase_a(1, hook=hook_a1, block_cb=a1_block_cb)

        def hook_b0():
            f1()
            bc11 = ln_finalize(*s1)
            normalize1(r1b1, h8_1, bc11)

        v0, s20, f20 = phase_b(0, r1b0, h8_0, hook=hook_b0)

        def hook_b1():
            f20()
            bc20 = ln_finalize(*s20)
            phase_c_out(0, v0, bc20, tail=False)

        v1, s21, f21 = phase_b(1, r1b1, h8_1, hook=hook_b1)
        f21()
        bc21 = ln_finalize(*s21)
        phase_c_out(1, v1, bc21, tail=True)

    nc.compile()
    return nc


def _get_nc():
    global _CACHED_NC
    if _CACHED_NC is None:
        _CACHED_NC = _build()
    return _CACHED_NC


def _prep_inputs(x, w_kv, b_kv, w_fc, b_fc, w_mproj, b_mproj,
                 ln1_g, ln1_b, ln2_g, ln2_b):
    """Host-side shard + retile. Returns per-core input maps."""
    bf = ml_dtypes.bfloat16
    x_flat = np.ascontiguousarray(np.asarray(x, dtype=np.float32).reshape(B * S, E))

    # weights: [in, out] -> [m, p, o, c] tiles, in = o*128+p, out = m*128+c
    def retile(w, io, oo):
        w = np.asarray(w, dtype=np.float32).reshape(io, P, oo, P)
        return np.ascontiguousarray(w.transpose(2, 1, 0, 3))

    wkv_t = retile(w_kv, EO, EO).astype(bf)
    wfc_t = retile(w_fc, EO, FO)     # [FO, P, EO, P] f32
    wmp_t = retile(w_mproj, FO, EO)  # [EO, P, FO, P] f32

    # bf16 part pre-scaled to the shared fp8 product scale; fp8 part
    # quantized at weight scale SW.
    wfc_bf = (wfc_t[:, :, :BFFC, :] * FC_SCALE).astype(bf)
    wmp_bf = (wmp_t[:, :, :BFMP, :] * MP_SCALE).astype(bf)

    def f8pairs(wslice, nch):
        # [oo, P, nch, P] -> [oo, P, nch//2, 2, P] fp8 at scale SW
        if nch == 0:
            oo = wslice.shape[0]
            return np.zeros((oo, P, 1, 2, P), E4NP)
        q = (wslice * SW).astype(E4NP)
        oo = q.shape[0]
        return np.ascontiguousarray(q.reshape(oo, P, nch // 2, 2, P))

    wmp_f8 = f8pairs(wmp_t[:, :, BFMP:, :], F8MP)
    wfc_f8 = f8pairs(wfc_t[:, :, BFFC:, :], F8FC)

    def p2d(v):  # [n*P] -> [P, n] with chunk o in column o
        v = np.asarray(v, np.float32)
        return np.ascontiguousarray(v.reshape(-1, P).T)

    g1c = p2d(ln1_g)
    b1c = p2d(ln1_b)
    shared = {
        "wkv": wkv_t, "wfc": wfc_bf, "wmp": wmp_bf,
        "wmp8": wmp_f8, "wfc8": wfc_f8,
        "bkv": p2d(b_kv), "bfc": p2d(b_fc), "bmp": p2d(b_mproj),
        "g1": g1c, "b1": b1c,
        "g1s": g1c * SA_H, "b1s": b1c * SA_H,
        "g2": p2d(ln2_g), "b2": p2d(ln2_b),
    }
    in_maps = []
    for c in range(NCORES):
        xT = np.ascontiguousarray(x_flat[c * TOK:(c + 1) * TOK].T)  # [E, TOK] f32
        in_maps.append({"xf": xT, "xb": xT.astype(bf), **shared})
    return in_maps


def _run(inputs, trace=False):
    nc = _get_nc()
    in_maps = _prep_inputs(
        inputs["x"], inputs["w_kv"], inputs["b_kv"], inputs["w_fc"],
        inputs["b_fc"], inputs["w_mproj"], inputs["b_mproj"],
        inputs["ln1_g"], inputs["ln1_b"], inputs["ln2_g"], inputs["ln2_b"])
    res = run_bass_kernel_spmd(nc, in_maps, core_ids=list(range(NCORES)),
                               trace=trace)
    outs = [np.asarray(res.results[c]["out"], dtype=np.float32).T
            for c in range(NCORES)]
    full = np.concatenate(outs, axis=0).reshape(B, S, E)
    return full, res


def kernel(**inputs) -> np.ndarray:
    out, _ = _run(inputs, trace=False)
    return out
